# revision 1
# baseline (speedup 1.0000x reference)
"""Multi-head attention TRN2 Bass kernel, head-sharded across 8 NeuronCores.

Problem: S=2048, E=1024, H=16 heads, dk=dv=64, fp32.
    Q = x @ Wq.T ; K = x @ Wk.T ; V = x @ Wv.T   (per-head slices)
    A_h = softmax(Q_h K_h^T / 8) V_h
    out = concat_h(A_h) @ Wo.T

Sharding: tensor-parallel over heads. Core i owns heads (2i, 2i+1), computes
its heads' Q/K/V projections over the full sequence, attention, and a partial
output projection against the matching 128-column slice of Wo. The 8 partial
[2048,1024] outputs are summed on the host (the gather/unshard step).

On-chip layout (everything "transposed" so no PE transposes of big tensors
are needed and softmax normalization rides the AV matmul as a ones column):
    xT   [E, S]   E on partitions (8 chunks of 128), bf16
    QT [128, S]: rows 0-63 head A, 64-127 head B  (dk on partitions)
    KT zero-padded per head so scores stay K=128 (single PE tile mode
    everywhere - mode switches drain the PE array)
    scores^T chunks [sk=128, sq] = kpad_h.T @ QT
    exp on ACT (scale=1/8 fused), bf16 out
    AV:  A^T[dv+1, sq] accumulated over chunks; lhsT = [V_chunk | ones],
         row 64 collects the softmax denominator for free
    outproj: lhsT = normalized A1^T [128, sq], rhs = WoT slice [128, E]

All matmul operands bf16 (empirically ~0.4% rel err vs fp64 reference,
fp32 PSUM accumulation throughout); fast 2-byte weight loads keep the PE
at ~1 cycle/row.
"""

import numpy as np
import ml_dtypes

import concourse.mybir as mybir
import concourse.tile as tile
from concourse import bacc
from concourse.bass_utils import run_bass_kernel_spmd

S, E, H, DK, DV = 2048, 1024, 16, 64, 64
NCORES = 8
HPC = H // NCORES          # heads per core = 2
CSL = HPC * DV             # concat-dim columns per core = 128
P = 128
NE = E // P                # 8 contraction chunks for projections
SQB = 512                  # sequence block (PSUM-bank-limited matmul width)
NSQB = S // SQB            # 4
NCH = S // P               # 16 sk chunks of 128
F32 = mybir.dt.float32
BF16 = mybir.dt.bfloat16
SCALE = 1.0 / np.sqrt(DK).astype(np.float32)  # 1/8

EXP = mybir.ActivationFunctionType.Exp
MULT = mybir.AluOpType.mult

_cache = {}
last_results = None  # BassKernelResults of the most recent run (for test.py)
TRACE = False


def _build_nc():
    nc = bacc.Bacc("TRN2", target_bir_lowering=False, debug=False)

    # host pre-arranges everything partition-major (and bf16) for fast DMA
    xT = nc.dram_tensor("xT", [P, NE, S], BF16, kind="ExternalInput")
    wqT = nc.dram_tensor("wqT", [P, NE, CSL], BF16, kind="ExternalInput")
    wkT = nc.dram_tensor("wkT", [P, NE, CSL], BF16, kind="ExternalInput")
    wvT = nc.dram_tensor("wvT", [P, NE, CSL], BF16, kind="ExternalInput")
    woT = nc.dram_tensor("woT", [CSL, E], BF16, kind="ExternalInput")
    y = nc.dram_tensor("y", [S, E], BF16, kind="ExternalOutput")

    xT_r = xT.ap()
    w_r = {"q": wqT.ap(), "k": wkT.ap(), "v": wvT.ap()}
    y_ap = y.ap()

    with tile.TileContext(nc) as tc:
        with tc.tile_pool(name="persist", bufs=1) as persist, \
             tc.tile_pool(name="xw", bufs=1) as xw:
            # Persistent SBUF tensors
            qt = persist.tile([P, S], BF16)          # QT, both heads stacked
            kpad = [
                persist.tile([P, S], BF16, name=f"kpad{h}", tag=f"kpad{h}")
                for h in range(HPC)
            ]
            vaug = [
                persist.tile([P, NCH, DV + 2], BF16, name=f"vaug{h}", tag=f"vaug{h}")
                for h in range(HPC)
            ]
            wosb = persist.tile([P, E], BF16)

            # zero the unused half of each per-head padded KT, set ones cols
            nc.gpsimd.memset(kpad[0][DK:P, :], 0.0)
            nc.gpsimd.memset(kpad[1][0:DK, :], 0.0)
            for h in range(HPC):
                nc.gpsimd.memset(vaug[h][:, :, DV : DV + 2], 1.0)

            nc.sync.dma_start(wosb[:], woT.ap())
            wsb = {}
            for m in ("k", "q", "v"):
                wsb[m] = xw.tile([P, NE, CSL], BF16, name=f"w{m}sb", tag=f"w{m}")
                nc.sync.dma_start(wsb[m][:], w_r[m][:])
            xsb = xw.tile([P, NE, S], BF16)
            qs = [nc.scalar, nc.gpsimd, nc.sync]
            for n in range(NE):
                qs[n % 3].dma_start(xsb[:, n, :], xT_r[:, n, :])

            # ---- Phase B: K/Q projections (KT/QT = W_slice^T.T @ x^T) ----
            with tc.tile_pool(name="proj_ps", bufs=2, space="PSUM") as proj_ps:
                for m in ("k", "q"):
                    for t in range(NSQB):
                        sl = slice(t * SQB, (t + 1) * SQB)
                        ps = proj_ps.tile([P, SQB], F32, tag="proj")
                        for n in range(NE):
                            nc.tensor.matmul(
                                ps[:], lhsT=wsb[m][:, n, :], rhs=xsb[:, n, sl],
                                start=(n == 0), stop=(n == NE - 1),
                            )
                        if m == "q":
                            nc.vector.tensor_copy(qt[:, sl], ps[:])
                        else:
                            nc.vector.tensor_copy(kpad[0][0:DK, sl], ps[0:DK, :])
                            nc.vector.tensor_copy(kpad[1][DK:P, sl], ps[DK:P, :])

            # ---- Phase D: attention + output projection, per sq block ----
            # V is computed on the fly during block 0 (emit_v_chunk), directly
            # in [sk, dv] orientation: V chunk c = x[128c:128c+128] @ Wv^T.
            with tc.tile_pool(name="sc_ps", bufs=2, space="PSUM") as sc_ps, \
                 tc.tile_pool(name="av_ps", bufs=3, space="PSUM") as av_ps, \
                 tc.tile_pool(name="op_ps", bufs=1, space="PSUM") as op_ps, \
                 tc.tile_pool(name="est", bufs=12) as est_pool, \
                 tc.tile_pool(name="a1t", bufs=2) as a1t_pool, \
                 tc.tile_pool(name="small", bufs=6) as small, \
                 tc.tile_pool(name="outp", bufs=6) as outp:

                def emit_v_chunk(c):
                    vp = op_ps.tile([P, P], F32, name="vp", tag="op")
                    for n in range(NE):
                        nc.tensor.matmul(
                            vp[:],
                            lhsT=xsb[:, n, c * P : (c + 1) * P],
                            rhs=wsb["v"][:, n, :],
                            start=(n == 0), stop=(n == NE - 1),
                        )
                    nc.vector.tensor_copy(vaug[0][:, c, 0:DV], vp[:, 0:DV])
                    nc.vector.tensor_copy(vaug[1][:, c, 0:DV], vp[:, DV:P])

                for b in range(NSQB):
                    bsl = slice(b * SQB, (b + 1) * SQB)
                    a1t = a1t_pool.tile([P, SQB], BF16, tag="a1t")
                    at_ps = [
                        av_ps.tile([P, SQB], F32, name=f"at_ps{h}", tag="av")
                        for h in range(HPC)
                    ]
                    for g in range(NCH // 2):
                        if b == 0:
                            emit_v_chunk(2 * g)
                            emit_v_chunk(2 * g + 1)
                        pss = [
                            sc_ps.tile([P, 2 * SQB], F32, name=f"scps{h}", tag="sc")
                            for h in range(HPC)
                        ]
                        for j in range(2):
                            c = 2 * g + j
                            for h in range(HPC):
                                nc.tensor.matmul(
                                    pss[h][:, j * SQB : (j + 1) * SQB],
                                    lhsT=kpad[h][:, c * P : (c + 1) * P],
                                    rhs=qt[:, bsl],
                                    start=True, stop=True,
                                )
                        ess = []
                        for h in range(HPC):
                            es = est_pool.tile(
                                [P, 2 * SQB], BF16, name=f"est{h}", tag="est"
                            )
                            nc.scalar.activation(
                                es[:], pss[h][:], EXP, scale=float(SCALE)
                            )
                            ess.append(es)
                        for j in range(2):
                            c = 2 * g + j
                            for h in range(HPC):
                                nc.tensor.matmul(
                                    at_ps[h][0 : DV + 2, :],
                                    lhsT=vaug[h][:, c, :],
                                    rhs=ess[h][:, j * SQB : (j + 1) * SQB],
                                    start=(c == 0), stop=(c == NCH - 1),
                                )
                    # normalize: A1T rows = A^T * (1/rowsum) broadcast
                    for h in range(HPC):
                        rs0 = small.tile([1, SQB], F32, tag="rs0")
                        nc.vector.tensor_copy(rs0[:], at_ps[h][DV : DV + 1, :])
                        rsr = small.tile([1, SQB], F32, tag="rsr")
                        nc.vector.reciprocal_approx_fast(rsr[:], rs0[:])
                        bc = small.tile([P, SQB], F32, tag="bc")
                        nc.gpsimd.partition_broadcast(bc[0:DV, :], rsr[:])
                        if h == 0:
                            nc.vector.tensor_tensor(
                                a1t[0:DV, :], at_ps[h][0:DV, :], bc[0:DV, :], MULT
                            )
                        else:
                            tb = small.tile([P, SQB], BF16, tag="tb")
                            nc.vector.tensor_tensor(
                                tb[0:DV, :], at_ps[h][0:DV, :], bc[0:DV, :], MULT
                            )
                            nc.gpsimd.dma_start(a1t[DV:P, :], tb[0:DV, :])

                    # output projection for this block
                    for j in range(NSQB):
                        rsl = slice(b * SQB + j * P, b * SQB + (j + 1) * P)
                        osb = outp.tile([P, E], BF16, tag="osb")
                        for e2 in range(E // SQB):
                            esl = slice(e2 * SQB, (e2 + 1) * SQB)
                            if b == NSQB - 1:
                                ops = sc_ps.tile(
                                    [P, SQB], F32, name="ops2", tag="sc"
                                )
                            else:
                                ops = op_ps.tile(
                                    [P, SQB], F32, name="ops", tag="op"
                                )
                            nc.tensor.matmul(
                                ops[:],
                                lhsT=a1t[:, j * P : (j + 1) * P],
                                rhs=wosb[:, esl],
                                start=True, stop=True,
                            )
                            nc.vector.tensor_copy(osb[:, esl], ops[:])
                        nc.sync.dma_start(y_ap[rsl, :], osb[:])

    nc.compile()
    return nc


def kernel(x, Wq, Wk, Wv, Wo):
    global last_results
    x = np.asarray(x, dtype=np.float32)
    Wq = np.asarray(Wq, dtype=np.float32)
    Wk = np.asarray(Wk, dtype=np.float32)
    Wv = np.asarray(Wv, dtype=np.float32)
    Wo = np.asarray(Wo, dtype=np.float32)

    if "nc" not in _cache:
        _cache["nc"] = _build_nc()
    nc = _cache["nc"]

    bf = ml_dtypes.bfloat16
    # [E, S] -> [P, NE, S] partition-major (chunk n, partition p = row n*P+p)
    xT = np.ascontiguousarray(
        x.T.reshape(NE, P, S).transpose(1, 0, 2).astype(bf)
    )
    WqT = np.ascontiguousarray(Wq.T)
    WkT = np.ascontiguousarray(Wk.T)
    WvT = np.ascontiguousarray(Wv.T)
    WoT = np.ascontiguousarray(Wo.T)

    in_maps = []
    for i in range(NCORES):
        sl = slice(i * CSL, (i + 1) * CSL)

        def wslice(WT):
            # [E, CSL] slice -> [P, NE, CSL] partition-major
            return np.ascontiguousarray(
                WT[:, sl].reshape(NE, P, CSL).transpose(1, 0, 2).astype(bf)
            )

        in_maps.append({
            "xT": xT,
            "wqT": wslice(WqT),
            "wkT": wslice(WkT),
            "wvT": wslice(WvT),
            "woT": np.ascontiguousarray(WoT[sl, :].astype(bf)),
        })

    last_results = run_bass_kernel_spmd(
        nc, in_maps, core_ids=list(range(NCORES)), trace=TRACE
    )
    out = np.zeros((S, E), dtype=np.float32)
    for r in last_results.results:
        out += r["y"].astype(np.float32)
    return out



# revision 2
# speedup vs baseline: 1.1289x; 1.1289x over previous
"""Multi-head attention TRN2 Bass kernel, head-sharded across 8 NeuronCores.

Problem: S=2048, E=1024, H=16 heads, dk=dv=64, fp32.
    Q = x @ Wq.T ; K = x @ Wk.T ; V = x @ Wv.T   (per-head slices)
    A_h = softmax(Q_h K_h^T / 8) V_h
    out = concat_h(A_h) @ Wo.T
Sharding: tensor-parallel over heads; core i owns heads (2i, 2i+1) and a
128-column slice of Wo. The 8 partial [2048,1024] outputs are summed on host.

Engine budget per core (the design drivers):
  - ACT exp is 2*S*S = 8.4M elems at 1 elem/lane/cycle @1.2GHz -> ~73us busy
    minimum (64 instrs of [128,1024]). It must start as early as possible and
    never starve.
  - PE: projections are full-K (E=1024) GEMMs; scores contract over dk=64
    only, so they run as two concurrent 64-row tile_position matmuls
    ((0,0)/(64,0)) -> both heads' scores in one 512-col stream span.
  - V is computed weight-stationary as V^T (8 LDWEIGHTS instead of 128) and
    DMA-transposed through the XBAR into the [sk, dv] layout AV needs.

Emission order = per-engine queue order, so the loop is software-pipelined:
  x arrives in per-sequence-block pieces; K/Q/V projections of block t+1 are
  interleaved into block 0's attention chunks (one of K/Q/V per chunk slot);
  scores(c+1) is emitted before AV(c) so the PE streams scores while ACT
  does exp(c); outproj of block b-1 is emitted inside block b's chunk loop.

Softmax normalization rides the AV matmul as ones-columns in the stationary
operand (rows 64/65 of the [66,512] PSUM accumulator collect the exp row
sums); normalization is a reciprocal+broadcast multiply on DVE/GPSIMD.

All matmul operands bf16 (fp32 PSUM accumulation; ~0.5% rel err).
"""

import numpy as np
import ml_dtypes

import concourse.mybir as mybir
import concourse.tile as tile
from concourse import bacc
from concourse.bass_utils import run_bass_kernel_spmd

S, E, H, DK, DV = 2048, 1024, 16, 64, 64
NCORES = 8
HPC = H // NCORES          # heads per core = 2
CSL = HPC * DV             # concat-dim columns per core = 128
P = 128
NE = E // P                # 8 contraction chunks for projections
SQB = 512                  # sequence block (PSUM-bank-limited matmul width)
NSQB = S // SQB            # 4
NCH = S // P               # 16 sk chunks of 128
F32 = mybir.dt.float32
BF16 = mybir.dt.bfloat16
SCALE = 1.0 / np.sqrt(DK).astype(np.float32)  # 1/8

EXP = mybir.ActivationFunctionType.Exp
MULT = mybir.AluOpType.mult

_cache = {}
last_results = None  # BassKernelResults of the most recent run (for test.py)
TRACE = False


def _build_nc():
    nc = bacc.Bacc("TRN2", target_bir_lowering=False, debug=False)

    # host pre-arranges everything partition-major (and bf16) for fast DMA
    xT = nc.dram_tensor("xT", [P, NE, S], BF16, kind="ExternalInput")
    wqT = nc.dram_tensor("wqT", [P, NE, CSL], BF16, kind="ExternalInput")
    wkT = nc.dram_tensor("wkT", [P, NE, CSL], BF16, kind="ExternalInput")
    wvT = nc.dram_tensor("wvT", [P, NE, CSL], BF16, kind="ExternalInput")
    woT = nc.dram_tensor("woT", [CSL, E], BF16, kind="ExternalInput")
    y = nc.dram_tensor("y", [S, E], BF16, kind="ExternalOutput")

    xT_r = xT.ap()
    w_r = {"q": wqT.ap(), "k": wkT.ap(), "v": wvT.ap()}
    y_ap = y.ap()

    with tile.TileContext(nc) as tc:
        with tc.tile_pool(name="persist", bufs=1) as persist, \
             tc.tile_pool(name="xw", bufs=1) as xw, \
             tc.tile_pool(name="proj_ps", bufs=2, space="PSUM") as proj_ps, \
             tc.tile_pool(name="sc_ps", bufs=2, space="PSUM") as sc_ps, \
             tc.tile_pool(name="at_ps", bufs=2, space="PSUM") as at_ps, \
             tc.tile_pool(name="est", bufs=6) as est_pool, \
             tc.tile_pool(name="a1t", bufs=2) as a1t_pool, \
             tc.tile_pool(name="small", bufs=8) as small, \
             tc.tile_pool(name="outp", bufs=4) as outp:

            # Persistent SBUF tensors. qt/kt: rows 0-63 head A (dk), 64-127
            # head B.  vtsb: V^T in the same layout.  vaug[h]: V chunks in
            # [sk, dv] + 2 ones columns (softmax denominator rows).
            qt = persist.tile([P, S], BF16)
            kt = persist.tile([P, S], BF16)
            vtsb = persist.tile([P, S], BF16)
            vaug = [
                persist.tile([P, NCH, DV + 2], BF16, name=f"vaug{h}", tag=f"vaug{h}")
                for h in range(HPC)
            ]
            wosb = persist.tile([P, E], BF16)

            for h in range(HPC):
                nc.gpsimd.memset(vaug[h][:, :, DV : DV + 2], 1.0)

            # Weights on sync queue first (small, needed by proj t0).
            nc.sync.dma_start(wosb[:], woT.ap())
            wsb = {}
            for m in ("k", "q", "v"):
                wsb[m] = xw.tile([P, NE, CSL], BF16, name=f"w{m}sb", tag=f"w{m}")
                nc.sync.dma_start(wsb[m][:], w_r[m][:])
            # x in per-(t, n) pieces, t-major so block 0 lands first.
            # t0 on sync (front of queue), rest on gpsimd.
            xsb = xw.tile([P, NE, S], BF16)
            for t in range(NSQB):
                tsl = slice(t * SQB, (t + 1) * SQB)
                q = nc.sync if t == 0 else nc.gpsimd
                for n in range(NE):
                    q.dma_start(xsb[:, n, tsl], xT_r[:, n, tsl])

            def emit_proj(m, t, dst):
                """One 512-col block of a projection, weight-stationary,
                accumulated over the 8 E-chunks; cast into dst (bf16)."""
                tsl = slice(t * SQB, (t + 1) * SQB)
                ps = proj_ps.tile([P, SQB], F32, tag="proj")
                for n in range(NE):
                    nc.tensor.matmul(
                        ps[:], lhsT=wsb[m][:, n, :], rhs=xsb[:, n, tsl],
                        start=(n == 0), stop=(n == NE - 1),
                    )
                nc.vector.tensor_copy(dst[:, tsl], ps[:])

            def emit_vtrans(t):
                """XBAR-transpose V^T block t into vaug[h][:, 4t:4t+4, 0:64].
                out[p, c, j] = vtsb[64h+j, 512t + 128c + p] = V_h[sk, j]."""
                for h in range(HPC):
                    nc.sync.dma_start_transpose(
                        vaug[h][:, 4 * t : 4 * t + 4, 0:DV],
                        vtsb[64 * h : 64 * h + 64, t * SQB : (t + 1) * SQB],
                    )

            # t0 projections up front (block 0 attention depends on them).
            emit_proj("k", 0, kt)
            emit_proj("q", 0, qt)
            emit_proj("v", 0, vtsb)
            emit_vtrans(0)

            def emit_scores(b, c):
                """Both heads' scores^T chunk in one span: two concurrent
                64-contraction-row tiles at tile_position (0,0)/(64,0)."""
                bsl = slice(b * SQB, (b + 1) * SQB)
                csl = slice(c * P, (c + 1) * P)
                sc = sc_ps.tile([P, 2 * SQB], F32, tag="sc")
                nc.tensor.matmul(
                    sc[:, 0:SQB], lhsT=kt[0:DK, csl], rhs=qt[0:DK, bsl],
                    start=True, stop=True,
                )
                nc.tensor.matmul(
                    sc[:, SQB : 2 * SQB], lhsT=kt[DK:P, csl], rhs=qt[DK:P, bsl],
                    start=True, stop=True,
                )
                return sc

            def emit_outproj(b, a1t):
                for j in range(NSQB):
                    rsl = slice(b * SQB + j * P, b * SQB + (j + 1) * P)
                    osb = outp.tile([P, E], BF16, tag="osb")
                    for e2 in range(E // SQB):
                        esl = slice(e2 * SQB, (e2 + 1) * SQB)
                        ops = proj_ps.tile([P, SQB], F32, tag="proj")
                        nc.tensor.matmul(
                            ops[:], lhsT=a1t[:, j * P : (j + 1) * P],
                            rhs=wosb[:, esl], start=True, stop=True,
                        )
                        nc.vector.tensor_copy(osb[:, esl], ops[:])
                    (nc.sync if j % 2 else nc.gpsimd).dma_start(y_ap[rsl, :], osb[:])

            def emit_normalize(at, a1t):
                """a1t rows = A^T * (1/rowsum); head B shifted to rows 64-127
                via gpsimd sbuf->sbuf DMA (lane-aligned ops can't cross
                partitions)."""
                for h in range(HPC):
                    rs0 = small.tile([1, SQB], F32, tag="rs0")
                    nc.vector.tensor_copy(rs0[:], at[h][DV : DV + 1, :])
                    rsr = small.tile([1, SQB], F32, tag="rsr")
                    nc.vector.reciprocal_approx_fast(rsr[:], rs0[:])
                    bc = small.tile([DV, SQB], F32, tag="bc")
                    nc.gpsimd.partition_broadcast(bc[:], rsr[:])
                    if h == 0:
                        nc.vector.tensor_tensor(
                            a1t[0:DV, :], at[h][0:DV, :], bc[:], MULT
                        )
                    else:
                        tb = small.tile([DV, SQB], BF16, tag="tb")
                        nc.vector.tensor_tensor(
                            tb[:], at[h][0:DV, :], bc[:], MULT
                        )
                        nc.gpsimd.dma_start(a1t[DV:P, :], tb[:])

            prev_a1t = None
            for b in range(NSQB):
                at = [
                    at_ps.tile([P, SQB], F32, name=f"at{h}", tag="at")
                    for h in range(HPC)
                ]
                a1t = a1t_pool.tile([P, SQB], BF16, tag="a1t")
                sc = emit_scores(b, 0)
                for c in range(NCH):
                    es = est_pool.tile([P, 2 * SQB], BF16, tag="est")
                    nc.scalar.activation(es[:], sc[:], EXP, scale=float(SCALE))
                    # interleave non-ACT-critical PE work behind the exp:
                    if b == 0 and c < 12:
                        t, r = c // 4 + 1, c % 4
                        if r == 0:
                            emit_proj("k", t, kt)
                        elif r == 1:
                            emit_proj("q", t, qt)
                        elif r == 2:
                            emit_proj("v", t, vtsb)
                            emit_vtrans(t)
                    if b > 0 and c == 2:
                        emit_outproj(b - 1, prev_a1t)
                    if c < NCH - 1:
                        sc = emit_scores(b, c + 1)
                    for h in range(HPC):
                        nc.tensor.matmul(
                            at[h][0 : DV + 2, :],
                            lhsT=vaug[h][:, c, :],
                            rhs=es[:, h * SQB : (h + 1) * SQB],
                            start=(c == 0), stop=(c == NCH - 1),
                        )
                emit_normalize(at, a1t)
                prev_a1t = a1t
            emit_outproj(NSQB - 1, prev_a1t)

    nc.compile()
    return nc


def kernel(x, Wq, Wk, Wv, Wo):
    global last_results
    x = np.asarray(x, dtype=np.float32)
    Wq = np.asarray(Wq, dtype=np.float32)
    Wk = np.asarray(Wk, dtype=np.float32)
    Wv = np.asarray(Wv, dtype=np.float32)
    Wo = np.asarray(Wo, dtype=np.float32)

    if "nc" not in _cache:
        _cache["nc"] = _build_nc()
    nc = _cache["nc"]

    bf = ml_dtypes.bfloat16
    # [E, S] -> [P, NE, S] partition-major (chunk n, partition p = row n*P+p)
    xT = np.ascontiguousarray(
        x.T.reshape(NE, P, S).transpose(1, 0, 2).astype(bf)
    )
    WqT = np.ascontiguousarray(Wq.T)
    WkT = np.ascontiguousarray(Wk.T)
    WvT = np.ascontiguousarray(Wv.T)
    WoT = np.ascontiguousarray(Wo.T)

    in_maps = []
    for i in range(NCORES):
        sl = slice(i * CSL, (i + 1) * CSL)

        def wslice(WT):
            # [E, CSL] slice -> [P, NE, CSL] partition-major
            return np.ascontiguousarray(
                WT[:, sl].reshape(NE, P, CSL).transpose(1, 0, 2).astype(bf)
            )

        in_maps.append({
            "xT": xT,
            "wqT": wslice(WqT),
            "wkT": wslice(WkT),
            "wvT": wslice(WvT),
            "woT": np.ascontiguousarray(WoT[sl, :].astype(bf)),
        })

    last_results = run_bass_kernel_spmd(
        nc, in_maps, core_ids=list(range(NCORES)), trace=TRACE
    )
    out = np.zeros((S, E), dtype=np.float32)
    for r in last_results.results:
        out += r["y"].astype(np.float32)
    return out
